# revision 1
# baseline (speedup 1.0000x reference)
"""Multi-head attention (B=2, S=4096, D=512, H=8, DR=64) on 8 trn2 NeuronCores.

Sharding: core c -> batch b = c // 4, head-pair hp = c % 4 (heads 2*hp, 2*hp+1).
Each core computes, for its batch and its two heads:
    q/k/v projections, flash-style attention (scores kept on-chip in
    transposed [t, s] orientation so softmax row-sums come from a fused
    ones-column in the AV matmul), and the partial output projection
    y_part = concat(out_h0, out_h1) @ Wo[rows of those heads].
Host sums the 4 partials per batch and adds the bias.

Matmul operands are cast to bf16 on-chip (fp32 matmuls on trn2 run as two
LOW/HIGH passes with an un-hidden LDWEIGHTS between them - ~3x the cost of a
bf16 matmul). All accumulation stays fp32 in PSUM; softmax denominators are
exact sums of the quantized bf16 weights, so attention rows still sum to 1.

The exp stream (33.5M elements/core, the single largest engine load) is
split between the Scalar engine (exact table exp, 5/8 of t-tiles) and the
Vector engine (3/8 of t-tiles, one-instruction Schraudolph: int16 bits of
the bf16 result, ~2% rms rel error that the denominator-consistency keeps
out of the row sums). The epilogue reciprocal runs as an int16 bit-trick
seed plus one fp32 Newton step on the broadcast tile (no slow exact
RECIPROCAL), and the per-block epilogue is staged (part1 / part2a / part2b,
deferred 14 and 24 t-tiles into the next block) so the in-order PE queue
never stalls on Vector-engine latency.

The input pipeline (x load/cast/transpose + q/k/v projections + v transpose)
is emitted in 8 groups of 512 s-columns, interleaved with the first
attention block's t-loop, so the exp stream starts after one group instead
of after the whole prologue. Per-group SBUF tiles give the Tile scheduler
the dataflow to overlap group g+1's production with attention over group g.
"""

import sys

for _p in ("/opt/trn_rl_repo", "/root/.axon_site/_ro/trn_rl_repo"):
    if _p not in sys.path:
        sys.path.insert(0, _p)

import numpy as np
from contextlib import ExitStack

import concourse.bass as bass
import concourse.tile as tile
import concourse.mybir as mybir
from concourse.bass_utils import run_bass_kernel_spmd
from concourse.masks import make_identity

B, S, D = 2, 4096, 512
H, DR = 8, 64
P = 128
NT = S // P          # 32 t-tiles (also s-tiles)
SBW = 512            # s-block width
NSB = S // SBW       # 8 s-blocks / t-groups
DC = D // P          # 4 d-chunks
GT = SBW // P        # 4 t-tiles per group
N_CORES = 8
FP32 = mybir.dt.float32
BF16 = mybir.dt.bfloat16
I16 = mybir.dt.int16

# exp offload: a subset of t-tiles compute exp on the Vector engine via a
# one-instruction Schraudolph (int16 bits of the bf16 result:
# bits = rint(score * EXP_A + EXP_B), bitcast to bf16 ~= exp(score/8) with
# ~2% rms relative error). The softmax denominator sums the same
# approximated values, so rows still sum to 1 and only the error
# *variation* reaches the output. Fraction tuned to balance ACT/DVE time
# against the tensor-engine roofline while keeping rel err << 2e-2.
EXP_A = float(128.0 / (np.sqrt(64.0) * np.log(2.0)))   # scale*128/ln2
EXP_B = float(127.0 * 128 - 5.625)                     # rint-optimal bias
RECIP_K = 32497.0   # bf16 reciprocal bit-trick magic (fp32-Newton-optimal)
DVE_EXP_RESIDUES = {1, 3, 6}   # of q % 8  -> alpha = 3/8
# how many of the 4 per-block y copies run on the Scalar engine
YSB_ON_ACT = 2

# This repo's walrus invocation hardcodes --enable-ldw-opt=false, which keeps
# every LDWEIGHTS serialized with its MATMUL (~380ns/MM instead of ~215).
# Rewrite the flag on the compiler command line.
def _patch_ldw_opt():
    from concourse import bass_utils as _bu

    if getattr(_bu, "_ldw_opt_patched", False):
        return
    _bu._ldw_opt_patched = True
    _orig = _bu.run_command

    def patched_run(argv, **kwargs):
        return _orig(argv, **kwargs)

    _bu.run_command = patched_run


_patch_ldw_opt()

_drain_patched = False


def _patch_tile_drain():
    """This walrus build rejects >1 sync wait on one instruction, which breaks
    TileContext's kernel-tail drain. Spread the waits over nop instructions
    emitted just before the drain."""
    global _drain_patched
    if _drain_patched:
        return
    _drain_patched = True

    def patched(self, tick_clock, wait_clock):
        nop0 = self.nc.sync.nop()
        wait_clock.add_sem_waits(
            nop0.ins, tile.ScopedClock({None: tick_clock.global_clock})
        )
        si = nop0.ins.sync_info
        waits = list(si.on_wait) if si is not None else []
        if waits:
            nop0.ins.sync_info = mybir.SyncInfo(on_wait=waits[:1], on_update=[])
            for w in waits[1:]:
                nop = self.nc.sync.nop()
                nop.ins.sync_info = mybir.SyncInfo(on_wait=[w], on_update=[])
        self.nc.sync.drain()
        self.nc.all_engine_barrier()
        popped = self.nc._tile_sem_poison_stack.pop()
        assert popped is self._sem_poison
        self.nc.clear_and_free_semaphores(list(self.sems.allocated().values()))
        self.nc.all_engine_barrier()

    tile.TileContext._drain_and_barrier = patched


# This walrus build supports only one sync-wait slot per instruction, while
# Tile's sem-assigner attaches up to ~3. Spread the excess onto NoOp
# instructions inserted immediately before the owning instruction (same
# engine, so the stall point is identical and no deadlock can be introduced).
_WAIT_LIMIT = 1
_SKIP_OPCODES = {"AllEngineBarrier", "EventSemaphore", "Call"}


def _split_sync_waits(nc: bass.Bass):
    noop_cls = getattr(mybir, "InstNoOp", None)
    if noop_cls is None:
        import bass_rust

        noop_cls = bass_rust.InstNoOp
    counter = [0]
    for f in nc.m.functions:
        for blk in f.blocks:
            insts = blk.instructions
            new_list = []
            changed = False
            for inst in insts:
                si = inst.sync_info
                waits = list(si.on_wait) if si is not None and si.on_wait else []
                if (
                    len(waits) > _WAIT_LIMIT
                    and inst.opcode not in _SKIP_OPCODES
                    and all(w.sync_type == "semaphore" for w in waits)
                ):
                    excess = waits[: len(waits) - _WAIT_LIMIT]
                    keep = waits[len(waits) - _WAIT_LIMIT :]
                    for w in excess:
                        counter[0] += 1
                        new_list.append(
                            noop_cls(
                                name=f"I-waitsplit-{counter[0]}",
                                engine=inst.engine,
                                debug=inst.debug,
                                ins=[],
                                outs=[],
                                sync_info=mybir.SyncInfo(
                                    on_wait=[w], on_update=[]
                                ),
                            )
                        )
                    inst.sync_info = mybir.SyncInfo(
                        on_wait=keep, on_update=list(si.on_update or [])
                    )
                    changed = True
                new_list.append(inst)
            if changed:
                insts.clear()
                insts.extend(new_list)


def _build_program() -> bass.Bass:
    _patch_tile_drain()
    nc = bass.Bass()

    xt_d = nc.declare_dram_parameter("xt", [D, S], BF16, isOutput=False)
    wq_d = nc.declare_dram_parameter("wq", [D, P], BF16, isOutput=False)
    wk_d = nc.declare_dram_parameter("wk", [D, P], BF16, isOutput=False)
    wv_d = nc.declare_dram_parameter("wv", [D, P], BF16, isOutput=False)
    wo_d = nc.declare_dram_parameter("wo", [P, D], BF16, isOutput=False)
    y_d = nc.declare_dram_parameter("y", [S, D], FP32, isOutput=True)

    with tile.TileContext(nc) as tc, ExitStack() as ctx:
        consts = ctx.enter_context(tc.tile_pool(name="consts", bufs=1))
        wpool = ctx.enter_context(tc.tile_pool(name="weights", bufs=1))
        big = ctx.enter_context(tc.tile_pool(name="big", bufs=1))
        aux = ctx.enter_context(tc.tile_pool(name="aux", bufs=2, space="PSUM"))
        psp = ctx.enter_context(tc.tile_pool(name="ps", bufs=2, space="PSUM"))
        pop = ctx.enter_context(tc.tile_pool(name="po", bufs=2, space="PSUM"))
        epool = ctx.enter_context(tc.tile_pool(name="exp", bufs=14))
        spool = ctx.enter_context(tc.tile_pool(name="small", bufs=4))
        opool = ctx.enter_context(tc.tile_pool(name="osb", bufs=3))
        ypool = ctx.enter_context(tc.tile_pool(name="yout", bufs=3))

        ones64 = consts.tile([1, 64], BF16)
        nc.vector.memset(ones64[:], 1.0)

        # PE warm-up: dense junk matmuls during the initial DMA-bound window
        # keep the HAM clock-gate at 8/8 so the first real matmuls run at
        # 2.4 GHz instead of 1.2 GHz.
        warm = consts.tile([P, D], BF16)
        nc.vector.memset(warm[:], 0.0)
        pw = aux.tile([P, D], FP32, tag="aux", name="pw")
        for _ in range(12):
            nc.tensor.matmul(
                pw[:], warm[:, 0:P], warm[:], start=True, stop=True
            )

        # Weights in bf16; w*_b[p, c*128 + e] = W[c*128 + p, e]
        wq_b = wpool.tile([P, D], BF16)
        wk_b = wpool.tile([P, D], BF16)
        wv_b = wpool.tile([P, D], BF16)
        wo_b = wpool.tile([P, D], BF16)
        for w_b, w_dram in ((wq_b, wq_d), (wk_b, wk_d), (wv_b, wv_d)):
            nc.gpsimd.dma_start(
                w_b[:].rearrange("p (c e) -> p c e", c=DC),
                w_dram[:].rearrange("(c p) e -> p c e", p=P),
            )
        nc.gpsimd.dma_start(wo_b[:], wo_d[:])

        # Per-group persistent tiles (bufs=NSB so every group stays live).
        # xT_g[g][p, c*512 + j] = x[g*512 + j, c*128 + p]
        xtp = ctx.enter_context(tc.tile_pool(name="xtg", bufs=NSB))
        ktp = ctx.enter_context(tc.tile_pool(name="ktg", bufs=NSB))
        qtp = ctx.enter_context(tc.tile_pool(name="qtg", bufs=NSB))
        vtp = ctx.enter_context(tc.tile_pool(name="vtg", bufs=2))
        vsp = ctx.enter_context(tc.tile_pool(name="vsg", bufs=NSB))
        xT_g = [None] * NSB
        kT_g = [None] * NSB   # [e(h0|h1), 512 t-cols]
        qT_g = [None] * NSB   # [e(h0|h1), 512 s-cols]
        v_g = [None] * NSB    # per t-tile in group: [t, 65*2] = [vh0|1 | vh1|1]

        def produce_x_q(g):
            xt = xtp.tile([P, DC * SBW], BF16, tag="xt")
            xT_g[g] = xt
            for c in range(DC):
                eng = nc.sync
                eng.dma_start(
                    xt[:, c * SBW : (c + 1) * SBW],
                    xt_d[c * P : (c + 1) * P, g * SBW : (g + 1) * SBW],
                )
            qt = qtp.tile([P, SBW], BF16, tag="qt")
            qT_g[g] = qt
            pp = aux.tile([P, SBW], FP32, tag="aux")
            for c in range(DC):
                nc.tensor.matmul(
                    pp[:],
                    wq_b[:, c * P : (c + 1) * P],
                    xt[:, c * SBW : (c + 1) * SBW],
                    start=(c == 0),
                    stop=(c == DC - 1),
                )
            nc.vector.tensor_copy(qt[:], pp[:])

        def produce_k(g):
            xt = xT_g[g]
            kt = ktp.tile([P, SBW], BF16, tag="kt")
            kT_g[g] = kt
            pp = aux.tile([P, SBW], FP32, tag="aux")
            for c in range(DC):
                nc.tensor.matmul(
                    pp[:],
                    wk_b[:, c * P : (c + 1) * P],
                    xt[:, c * SBW : (c + 1) * SBW],
                    start=(c == 0),
                    stop=(c == DC - 1),
                )
            nc.vector.tensor_copy(kt[:], pp[:])

        def produce_v_half(g, half):
            xt = xT_g[g]
            if half == 0:
                vs = vsp.tile([P, GT * 130], BF16, tag="vs")
                v_g[g] = vs
            else:
                vs = v_g[g]
            for j in (0, 1) if half == 0 else (2, 3):
                pv = aux.tile([P, P], FP32, tag="aux")
                for c in range(DC):
                    nc.tensor.matmul(
                        pv[:],
                        xt[:, c * SBW + j * P : c * SBW + (j + 1) * P],
                        wv_b[:, c * P : (c + 1) * P],
                        start=(c == 0),
                        stop=(c == DC - 1),
                    )
                dstv = vs[:, j * 130 : j * 130 + 130].rearrange(
                    "p (h q) -> p h q", h=2
                )[:, :, 0:64]
                nc.vector.tensor_copy(
                    dstv, pv[:].rearrange("p (h q) -> p h q", h=2)
                )
            if half == 1:
                ones_cols = vs[:].rearrange("p (t q) -> p t q", t=GT)[
                    :, :, 64:130:65
                ]
                nc.vector.memset(ones_cols, 1.0)

        def produce_group(g):
            produce_x_q(g)
            produce_k(g)
            produce_v_half(g, 0)
            produce_v_half(g, 1)

        # ---- attention + output projection ----
        # Epilogue part 1 (right after a block's t-loop): evacuate each head's
        # unnormalized output AND its softmax-sum row in one 65-partition bf16
        # copy, then kick a DMA transpose of the two sum rows into a
        # [128, 8] layout (s on partitions). Part 2 (deferred into the next
        # block's t-loop): tiny exact reciprocal on [128, 8], DMA-transpose
        # back to rows, GpSimd partition-broadcast to 128 partitions,
        # normalize, project.
        DEFER_ITERS = 14
        pending = [None]

        def epilogue_part1(sb, po0, po1):
            u0 = opool.tile([64, SBW], BF16, tag="u0")
            u1 = opool.tile([P, SBW], BF16, tag="u1")
            nc.vector.tensor_copy(u0[:], po0[0:64, :])
            # u1 e-rows at partitions 64..127, aligned with the bc slice in
            # the normalize mul
            nc.vector.tensor_copy(u1[64:128, :], po1[0:64, :])
            # sums rows to partition-0 tiles (matmul rhs must start at 0)
            s0 = spool.tile([1, SBW], BF16, tag="s0")
            s1 = spool.tile([1, SBW], BF16, tag="s1")
            nc.vector.tensor_copy(s0[:], po0[64:65, :])
            nc.vector.tensor_copy(s1[:], po1[64:65, :])
            pending[0] = (sb, u0, u1, s0, s1)

        epi_ops = []

        def epilogue_part2a():
            if pending[0] is None:
                return
            sb, u0, u1, s0, s1 = pending[0]
            # broadcast the bf16 sums rows to 128 partitions (1-pass bf16 MMs)
            pb_t = aux.tile([P, SBW], FP32, tag="aux")
            nc.tensor.matmul(
                pb_t[0:64, :], ones64[:], s0[:],
                start=True, stop=True, tile_position=(0, 0),
            )
            nc.tensor.matmul(
                pb_t[64:128, :], ones64[:], s1[:],
                start=True, stop=True, tile_position=(0, 64),
            )
            st8 = {}

            def op_bc():
                st8["bc"] = spool.tile([P, SBW], BF16, tag="bc", name="bc")
                nc.vector.tensor_copy(st8["bc"][:], pb_t[:])

            def op_seed():
                st8["rs"] = spool.tile([P, SBW], BF16, tag="rs", name="rs")
                nc.vector.tensor_scalar(
                    st8["rs"][:].bitcast(I16), st8["bc"][:].bitcast(I16),
                    -1.0, float(RECIP_K),
                    mybir.AluOpType.mult, mybir.AluOpType.add,
                )

            def op_t():
                st8["tt"] = spool.tile([P, SBW], FP32, tag="tt", name="tt")
                nc.vector.tensor_mul(st8["tt"][:], st8["bc"][:], st8["rs"][:])

            def op_u():
                st8["uu"] = spool.tile([P, SBW], FP32, tag="uu", name="uu")
                nc.vector.tensor_scalar(
                    st8["uu"][:], st8["tt"][:], -1.0, 2.0,
                    mybir.AluOpType.mult, mybir.AluOpType.add,
                )

            def op_rcb():
                st8["rcb"] = spool.tile([P, SBW], FP32, tag="rcb", name="rcb")
                nc.vector.tensor_mul(st8["rcb"][:], st8["rs"][:], st8["uu"][:])

            def op_mul():
                st8["osb"] = spool.tile([P, SBW], BF16, tag="osb2", name="osb2")
                nc.vector.tensor_mul(
                    st8["osb"][0:64, :], u0[:], st8["rcb"][0:64, :]
                )
                nc.vector.tensor_mul(
                    st8["osb"][64:128, :], u1[64:128, :], st8["rcb"][64:128, :]
                )

            def mk_proj(st):
                def op_proj():
                    sl = slice(st * P, (st + 1) * P)
                    py_t = aux.tile([P, D], FP32, tag="aux", name="pyt")
                    nc.tensor.matmul(
                        py_t[:], st8["osb"][:, sl], wo_b[:],
                        start=True, stop=True,
                    )
                    ysb = ypool.tile([P, D], FP32, tag="y", name="ysb")
                    if st < YSB_ON_ACT:
                        nc.scalar.copy(ysb[:], py_t[:])
                    else:
                        nc.vector.tensor_copy(ysb[:], py_t[:])
                    row = (sb * (SBW // P) + st) * P
                    nc.sync.dma_start(y_d[row : row + P, :], ysb[:])
                return op_proj

            epi_ops.extend(
                [op_bc, op_seed, op_t, op_u, op_rcb, op_mul]
                + [mk_proj(st) for st in range(SBW // P)]
            )
            pending[0] = None

        produce_group(0)
        produce_group(1)

        PREF = 8
        SPLICE = {}
        for _g in range(2, NSB):
            base = 2 + (_g - 2) * 4
            SPLICE[base] = lambda g=_g: produce_x_q(g)
            SPLICE[base + 1] = lambda g=_g: produce_k(g)
            SPLICE[base + 2] = lambda g=_g: produce_v_half(g, 0)
            SPLICE[base + 3] = lambda g=_g: produce_v_half(g, 1)
        NQ = NSB * NT
        po_cur = [None, None]
        ex_q = {}
        for q in range(NQ + PREF):
            if q < NQ:
                sb, tt = q // NT, q % NT
                g, j = tt // GT, tt % GT
                if sb == 0 and tt in SPLICE:
                    SPLICE[tt]()
            # AV of 8 tiles ago is emitted BEFORE this tile's scores: its
            # inputs are long ready, so the in-order PE queue keeps
            # streaming even when the scores' exp dependency lags.
            if q >= PREF:
                qa = q - PREF
                sba, ta = qa // NT, qa % NT
                ga, ja = ta // GT, ta % GT
                if ta == 0:
                    po_cur[0] = pop.tile([65, SBW], FP32, tag="po", name="po0")
                    po_cur[1] = pop.tile([65, SBW], FP32, tag="po", name="po1")
                po0, po1 = po_cur
                vs, ex = v_g[ga], ex_q.pop(qa)
                nc.tensor.matmul(
                    po0[:],
                    vs[:, ja * 130 : ja * 130 + 65],
                    ex[:, 0:SBW],
                    start=(ta == 0),
                    stop=(ta == NT - 1),
                )
                nc.tensor.matmul(
                    po1[:],
                    vs[:, ja * 130 + 65 : ja * 130 + 130],
                    ex[:, SBW : 2 * SBW],
                    start=(ta == 0),
                    stop=(ta == NT - 1),
                )
                if ta == NT - 1:
                    epilogue_part1(sba, po0, po1)
            if q < NQ:
                kt, qt = kT_g[g], qT_g[sb]
                ps_t = psp.tile([P, 2 * SBW], FP32, tag="ps")
                nc.tensor.matmul(
                    ps_t[:, 0:SBW],
                    kt[0:64, j * P : (j + 1) * P],
                    qt[0:64, :],
                    start=True,
                    stop=True,
                    tile_position=(0, 0),
                )
                nc.tensor.matmul(
                    ps_t[:, SBW : 2 * SBW],
                    kt[64:128, j * P : (j + 1) * P],
                    qt[64:128, :],
                    start=True,
                    stop=True,
                    tile_position=(64, 0),
                )
                ex = epool.tile([P, 2 * SBW], BF16, tag="exp")
                if (q % 8) in DVE_EXP_RESIDUES:
                    nc.vector.tensor_scalar(
                        ex[:].bitcast(I16), ps_t[:], EXP_A, EXP_B,
                        mybir.AluOpType.mult, mybir.AluOpType.add,
                    )
                else:
                    nc.scalar.activation(
                        ex[:], ps_t[:], mybir.ActivationFunctionType.Exp,
                        scale=float(1.0 / np.sqrt(DR)),
                    )
                ex_q[q] = ex
                if tt == DEFER_ITERS:
                    epilogue_part2a()
                if epi_ops and tt % 2 == 0:
                    epi_ops.pop(0)()
        epilogue_part2a()
        while epi_ops:
            epi_ops.pop(0)()

    _split_sync_waits(nc)
    return nc


_program = None


def _get_program():
    global _program
    if _program is None:
        _program = _build_program()
    return _program


def _make_in_maps(x, Wq, Wk, Wv, Wo):
    import ml_dtypes

    bf16 = ml_dtypes.bfloat16
    xts = [np.ascontiguousarray(x[b].T).astype(bf16) for b in range(B)]
    in_maps = []
    for c in range(N_CORES):
        b = c // 4
        hp = c % 4
        h0, h1 = 2 * hp, 2 * hp + 1
        in_maps.append(
            {
                "xt": xts[b],
                "wq": np.ascontiguousarray(
                    np.concatenate([Wq[h0], Wq[h1]], axis=1)
                ).astype(bf16),
                "wk": np.ascontiguousarray(
                    np.concatenate([Wk[h0], Wk[h1]], axis=1)
                ).astype(bf16),
                "wv": np.ascontiguousarray(
                    np.concatenate([Wv[h0], Wv[h1]], axis=1)
                ).astype(bf16),
                "wo": np.ascontiguousarray(Wo[hp * 128 : (hp + 1) * 128]).astype(
                    bf16
                ),
            }
        )
    return in_maps


def kernel(**inputs) -> np.ndarray:
    x = np.asarray(inputs["x"], dtype=np.float32)
    Wq = np.asarray(inputs["Wq"], dtype=np.float32)
    Wk = np.asarray(inputs["Wk"], dtype=np.float32)
    Wv = np.asarray(inputs["Wv"], dtype=np.float32)
    Wo = np.asarray(inputs["Wo"], dtype=np.float32)
    bo = np.asarray(inputs["bo"], dtype=np.float32)

    nc = _get_program()
    in_maps = _make_in_maps(x, Wq, Wk, Wv, Wo)
    res = run_bass_kernel_spmd(nc, in_maps, list(range(N_CORES)))

    y = np.zeros((B, S, D), dtype=np.float32)
    for c in range(N_CORES):
        y[c // 4] += res.results[c]["y"]
    y += bo[None, None, :]
    return y



# revision 6
# speedup vs baseline: 1.0084x; 1.0084x over previous
"""Multi-head attention (B=2, S=4096, D=512, H=8, DR=64) on 8 trn2 NeuronCores.

Sharding: core c -> batch b = c // 4, head-pair hp = c % 4 (heads 2*hp, 2*hp+1).
Each core computes, for its batch and its two heads:
    q/k/v projections, flash-style attention (scores kept on-chip in
    transposed [t, s] orientation so softmax row-sums come from a fused
    ones-column in the AV matmul), and the partial output projection
    y_part = concat(out_h0, out_h1) @ Wo[rows of those heads].
Host sums the 4 partials per batch and adds the bias.

Matmul operands are cast to bf16 on-chip (fp32 matmuls on trn2 run as two
LOW/HIGH passes with an un-hidden LDWEIGHTS between them - ~3x the cost of a
bf16 matmul). All accumulation stays fp32 in PSUM; softmax denominators are
sums of the same quantized bf16 weights used in the AV matmul, keeping the
row-stochastic error common-mode.

Main-loop emission is batched two t-tiles at a time (scores pair-pair, then
the AV pairs of 2 earlier tiles): the AV stationary spans all 128 PE rows,
so its LDWEIGHTS cannot be pulled ahead past in-flight scores matmuls
(row-group conflict) and every scores<->AV transition exposes ~100ns of
weight-load; batching halves the transition count. The scores pair itself
runs concurrently on disjoint row groups (tile_position (0,0)/(64,0)), and
the per-head AV matmuls (M=65: 64 v-dims + the fused ones column) serialize
- 130 stationary columns don't fit the 128-wide array, which is structural.

The exp stream (33.5M elements/core, the largest engine load) is split
between the Scalar engine (exact table exp) and the Vector engine
(one-instruction Schraudolph: int16 bits of the bf16 result, ~2% rms rel
error whose row-sum component cancels against the fused denominator).
The per-block epilogue keeps the Vector queue shallow near block
boundaries: unnormalized-output evacuation is split ACT/DVE, the
reciprocal runs as an int16 bit-trick seed plus one bf16 Newton step, and
all epilogue ops are staged two t-tiles apart inside the next block's
t-loop so the in-order PE queue never stalls on engine latency.

The input pipeline (x load/cast + q/k/v projections) is emitted in 8 groups
of 512 s-columns; group-0/1 DMAs fan out across four engine queues so the
first projection matmul can issue ~4us earlier than a single-queue load.
"""

import sys

for _p in ("/opt/trn_rl_repo", "/root/.axon_site/_ro/trn_rl_repo"):
    if _p not in sys.path:
        sys.path.insert(0, _p)

import numpy as np
from contextlib import ExitStack

import concourse.bass as bass
import concourse.tile as tile
import concourse.mybir as mybir
from concourse.bass_utils import run_bass_kernel_spmd

B, S, D = 2, 4096, 512
H, DR = 8, 64
P = 128
NT = S // P          # 32 t-tiles (also s-tiles)
SBW = 512            # s-block width
NSB = S // SBW       # 8 s-blocks / t-groups
DC = D // P          # 4 d-chunks
GT = SBW // P        # 4 t-tiles per group
N_CORES = 8
FP32 = mybir.dt.float32
BF16 = mybir.dt.bfloat16
I16 = mybir.dt.int16

# exp offload: a subset of t-tiles compute exp on the Vector engine via a
# one-instruction Schraudolph (int16 bits of the bf16 result:
# bits = rint(score * EXP_A + EXP_B), bitcast to bf16 ~= exp(score/8) with
# ~2% rms relative error). The softmax denominator sums the same
# approximated values, so rows still sum to 1 and only the error
# *variation* reaches the output. Fraction tuned to balance ACT/DVE time
# against the tensor-engine roofline while keeping rel err << 2e-2.
EXP_A = float(128.0 / (np.sqrt(64.0) * np.log(2.0)))   # scale*128/ln2
EXP_B = float(127.0 * 128 - 5.625)                     # rint-optimal bias
RECIP_K = 32497.0   # bf16 reciprocal bit-trick magic (Newton-optimal)
DVE_EXP_RESIDUES = {1, 3, 6, 9, 11, 14}   # of q % 16  -> alpha = 6/16
# how many of the 4 per-block y copies run on the Scalar engine
YSB_ON_ACT = 2
PREF = 6             # AV lags scores by PREF t-tiles
DEFER_ITERS = 14     # epilogue part2 starts this many tiles into next block


# Kept as an extension point: this repo's walrus invocation hardcodes
# --enable-ldw-opt=false; flipping it to true crashes walrus codegen
# (visitInstLdweights), so the serialized weight-load cost is structural.
def _patch_ldw_opt():
    from concourse import bass_utils as _bu

    if getattr(_bu, "_ldw_opt_patched", False):
        return
    _bu._ldw_opt_patched = True
    _orig = _bu.run_command

    def patched_run(argv, **kwargs):
        return _orig(argv, **kwargs)

    _bu.run_command = patched_run


_patch_ldw_opt()

_drain_patched = False


def _patch_tile_drain():
    """This walrus build rejects >1 sync wait on one instruction, which breaks
    TileContext's kernel-tail drain. Spread the waits over nop instructions
    emitted just before the drain."""
    global _drain_patched
    if _drain_patched:
        return
    _drain_patched = True

    def patched(self, tick_clock, wait_clock):
        nop0 = self.nc.sync.nop()
        wait_clock.add_sem_waits(
            nop0.ins, tile.ScopedClock({None: tick_clock.global_clock})
        )
        si = nop0.ins.sync_info
        waits = list(si.on_wait) if si is not None else []
        if waits:
            nop0.ins.sync_info = mybir.SyncInfo(on_wait=waits[:1], on_update=[])
            for w in waits[1:]:
                nop = self.nc.sync.nop()
                nop.ins.sync_info = mybir.SyncInfo(on_wait=[w], on_update=[])
        self.nc.sync.drain()
        self.nc.all_engine_barrier()
        popped = self.nc._tile_sem_poison_stack.pop()
        assert popped is self._sem_poison
        self.nc.clear_and_free_semaphores(list(self.sems.allocated().values()))
        self.nc.all_engine_barrier()

    tile.TileContext._drain_and_barrier = patched


# This walrus build supports only one sync-wait slot per instruction, while
# Tile's sem-assigner attaches up to ~3. Spread the excess onto NoOp
# instructions inserted immediately before the owning instruction (same
# engine, so the stall point is identical and no deadlock can be introduced).
_WAIT_LIMIT = 1
_SKIP_OPCODES = {"AllEngineBarrier", "EventSemaphore", "Call"}


def _split_sync_waits(nc: bass.Bass):
    noop_cls = getattr(mybir, "InstNoOp", None)
    if noop_cls is None:
        import bass_rust

        noop_cls = bass_rust.InstNoOp
    counter = [0]
    for f in nc.m.functions:
        for blk in f.blocks:
            insts = blk.instructions
            new_list = []
            changed = False
            for inst in insts:
                si = inst.sync_info
                waits = list(si.on_wait) if si is not None and si.on_wait else []
                if (
                    len(waits) > _WAIT_LIMIT
                    and inst.opcode not in _SKIP_OPCODES
                    and all(w.sync_type == "semaphore" for w in waits)
                ):
                    excess = waits[: len(waits) - _WAIT_LIMIT]
                    keep = waits[len(waits) - _WAIT_LIMIT :]
                    for w in excess:
                        counter[0] += 1
                        new_list.append(
                            noop_cls(
                                name=f"I-waitsplit-{counter[0]}",
                                engine=inst.engine,
                                debug=inst.debug,
                                ins=[],
                                outs=[],
                                sync_info=mybir.SyncInfo(
                                    on_wait=[w], on_update=[]
                                ),
                            )
                        )
                    inst.sync_info = mybir.SyncInfo(
                        on_wait=keep, on_update=list(si.on_update or [])
                    )
                    changed = True
                new_list.append(inst)
            if changed:
                insts.clear()
                insts.extend(new_list)


def _build_program() -> bass.Bass:
    _patch_tile_drain()
    nc = bass.Bass()

    xt_d = nc.declare_dram_parameter("xt", [D, S], BF16, isOutput=False)
    wq_d = nc.declare_dram_parameter("wq", [D, P], BF16, isOutput=False)
    wk_d = nc.declare_dram_parameter("wk", [D, P], BF16, isOutput=False)
    wv_d = nc.declare_dram_parameter("wv", [D, P], BF16, isOutput=False)
    wo_d = nc.declare_dram_parameter("wo", [P, D], BF16, isOutput=False)
    y_d = nc.declare_dram_parameter("y", [S, D], FP32, isOutput=True)

    with tile.TileContext(nc) as tc, ExitStack() as ctx:
        consts = ctx.enter_context(tc.tile_pool(name="consts", bufs=1))
        wpool = ctx.enter_context(tc.tile_pool(name="weights", bufs=1))
        aux = ctx.enter_context(tc.tile_pool(name="aux", bufs=2, space="PSUM"))
        psp = ctx.enter_context(tc.tile_pool(name="ps", bufs=2, space="PSUM"))
        pop = ctx.enter_context(tc.tile_pool(name="po", bufs=2, space="PSUM"))
        epool = ctx.enter_context(tc.tile_pool(name="exp", bufs=10))
        spool = ctx.enter_context(tc.tile_pool(name="small", bufs=4))
        opool = ctx.enter_context(tc.tile_pool(name="osb", bufs=3))
        ypool = ctx.enter_context(tc.tile_pool(name="yout", bufs=3))

        # Weights in bf16; w*_b[p, c*128 + e] = W[c*128 + p, e].  wq first so
        # the first q-projection matmul can issue as early as possible.
        wq_b = wpool.tile([P, D], BF16)
        wk_b = wpool.tile([P, D], BF16)
        wv_b = wpool.tile([P, D], BF16)
        wo_b = wpool.tile([P, D], BF16)
        nc.gpsimd.dma_start(
            wq_b[:].rearrange("p (c e) -> p c e", c=DC),
            wq_d[:].rearrange("(c p) e -> p c e", p=P),
        )

        # Per-group persistent tiles (bufs=NSB so every group stays live).
        # xT_g[g][p, c*512 + j] = x[g*512 + j, c*128 + p]
        xtp = ctx.enter_context(tc.tile_pool(name="xtg", bufs=NSB))
        ktp = ctx.enter_context(tc.tile_pool(name="ktg", bufs=NSB))
        qtp = ctx.enter_context(tc.tile_pool(name="qtg", bufs=NSB))
        vsp = ctx.enter_context(tc.tile_pool(name="vsg", bufs=NSB))
        xT_g = [None] * NSB
        kT_g = [None] * NSB   # [e(h0|h1), 512 t-cols]
        qT_g = [None] * NSB   # [e(h0|h1), 512 s-cols]
        v_g = [None] * NSB    # per t-tile in group: [t, 65*2] = [vh0|1 | vh1|1]

        # group-0 x chunks fan out over four queues (prologue-idle engines)
        _G0_ENGINES = None

        def produce_x(g):
            xt = xtp.tile([P, DC * SBW], BF16, tag="xt")
            xT_g[g] = xt
            for c in range(DC):
                eng = (
                    _G0_ENGINES[c] if g == 0 else (nc.sync if c % 2 else nc.gpsimd)
                )
                eng.dma_start(
                    xt[:, c * SBW : (c + 1) * SBW],
                    xt_d[c * P : (c + 1) * P, g * SBW : (g + 1) * SBW],
                )

        def produce_q(g):
            xt = xT_g[g]
            qt = qtp.tile([P, SBW], BF16, tag="qt")
            qT_g[g] = qt
            pp = aux.tile([P, SBW], FP32, tag="aux")
            for c in range(DC):
                nc.tensor.matmul(
                    pp[:],
                    wq_b[:, c * P : (c + 1) * P],
                    xt[:, c * SBW : (c + 1) * SBW],
                    start=(c == 0),
                    stop=(c == DC - 1),
                )
            nc.vector.tensor_copy(qt[:], pp[:])

        def produce_x_q(g):
            produce_x(g)
            produce_q(g)

        def produce_k(g):
            xt = xT_g[g]
            kt = ktp.tile([P, SBW], BF16, tag="kt")
            kT_g[g] = kt
            pp = aux.tile([P, SBW], FP32, tag="aux")
            for c in range(DC):
                nc.tensor.matmul(
                    pp[:],
                    wk_b[:, c * P : (c + 1) * P],
                    xt[:, c * SBW : (c + 1) * SBW],
                    start=(c == 0),
                    stop=(c == DC - 1),
                )
            nc.vector.tensor_copy(kt[:], pp[:])

        def produce_v_half(g, half):
            xt = xT_g[g]
            if half == 0:
                vs = vsp.tile([P, GT * 130], BF16, tag="vs")
                v_g[g] = vs
            else:
                vs = v_g[g]
            for j in (0, 1) if half == 0 else (2, 3):
                pv = aux.tile([P, P], FP32, tag="aux")
                for c in range(DC):
                    nc.tensor.matmul(
                        pv[:],
                        xt[:, c * SBW + j * P : c * SBW + (j + 1) * P],
                        wv_b[:, c * P : (c + 1) * P],
                        start=(c == 0),
                        stop=(c == DC - 1),
                    )
                dstv = vs[:, j * 130 : j * 130 + 130].rearrange(
                    "p (h q) -> p h q", h=2
                )[:, :, 0:64]
                nc.vector.tensor_copy(
                    dstv, pv[:].rearrange("p (h q) -> p h q", h=2)
                )
            if half == 1:
                ones_cols = vs[:].rearrange("p (t q) -> p t q", t=GT)[
                    :, :, 64:130:65
                ]
                nc.vector.memset(ones_cols, 1.0)

        # ---- prologue: group 0/1 inputs + weights, maximally parallel ----
        _G0_ENGINES = [nc.sync, nc.scalar, nc.sync, nc.gpsimd]
        produce_x(0)
        nc.scalar.dma_start(
            wk_b[:].rearrange("p (c e) -> p c e", c=DC),
            wk_d[:].rearrange("(c p) e -> p c e", p=P),
        )
        produce_x(1)
        nc.scalar.dma_start(
            wv_b[:].rearrange("p (c e) -> p c e", c=DC),
            wv_d[:].rearrange("(c p) e -> p c e", p=P),
        )
        nc.scalar.dma_start(wo_b[:], wo_d[:])

        ones64 = consts.tile([1, 64], BF16)
        nc.vector.memset(ones64[:], 1.0)

        produce_q(0)
        produce_k(0)
        produce_v_half(0, 0)
        produce_v_half(0, 1)
        produce_q(1)
        produce_k(1)
        produce_v_half(1, 0)
        produce_v_half(1, 1)

        # ---- attention + output projection ----
        # Epilogue part 1 (right after a block's t-loop): evacuate both heads'
        # unnormalized output into one [128, 512] bf16 tile (h1 at partitions
        # 64..127, aligned with the broadcast slice used by the normalize
        # mul), plus the two softmax-sum rows. Part 2 (deferred into the next
        # block's t-loop): broadcast the sums to 128 partitions with two tiny
        # col-tiled matmuls, reciprocal via int16 bit-trick seed + one bf16
        # Newton step, one normalize mul, then the four projection matmuls.
        pending = [None]

        def epilogue_part1(sb, po0, po1):
            u01 = opool.tile([P, SBW], BF16, tag="u01")
            nc.scalar.copy(u01[0:64, :], po0[0:64, :])
            nc.vector.tensor_copy(u01[64:128, :], po1[0:64, :])
            # sums rows to partition-0 tiles (matmul rhs must start at 0)
            s0 = spool.tile([1, SBW], BF16, tag="s0")
            s1 = spool.tile([1, SBW], BF16, tag="s1")
            nc.vector.tensor_copy(s0[:], po0[64:65, :])
            nc.vector.tensor_copy(s1[:], po1[64:65, :])
            pending[0] = (sb, u01, s0, s1)

        epi_ops = []

        def epilogue_part2a():
            if pending[0] is None:
                return
            sb, u01, s0, s1 = pending[0]
            # broadcast the bf16 sums rows to 128 partitions (1-pass bf16 MMs)
            pb_t = aux.tile([P, SBW], FP32, tag="aux")
            nc.tensor.matmul(
                pb_t[0:64, :], ones64[:], s0[:],
                start=True, stop=True, tile_position=(0, 0),
            )
            nc.tensor.matmul(
                pb_t[64:128, :], ones64[:], s1[:],
                start=True, stop=True, tile_position=(0, 64),
            )
            st8 = {}

            def op_bc():
                st8["bc"] = spool.tile([P, SBW], BF16, tag="bc", name="bc")
                nc.scalar.copy(st8["bc"][:], pb_t[:])

            def op_seed():
                st8["rs"] = spool.tile([P, SBW], BF16, tag="rs", name="rs")
                nc.vector.tensor_scalar(
                    st8["rs"][:].bitcast(I16), st8["bc"][:].bitcast(I16),
                    -1.0, float(RECIP_K),
                    mybir.AluOpType.mult, mybir.AluOpType.add,
                )

            def op_t():
                st8["tt"] = spool.tile([P, SBW], BF16, tag="tt", name="tt")
                nc.vector.tensor_mul(st8["tt"][:], st8["bc"][:], st8["rs"][:])

            def op_u():
                st8["uu"] = spool.tile([P, SBW], BF16, tag="uu", name="uu")
                nc.vector.tensor_scalar(
                    st8["uu"][:], st8["tt"][:], -1.0, 2.0,
                    mybir.AluOpType.mult, mybir.AluOpType.add,
                )

            def op_rcb():
                st8["rcb"] = spool.tile([P, SBW], BF16, tag="rcb", name="rcb")
                nc.vector.tensor_mul(st8["rcb"][:], st8["rs"][:], st8["uu"][:])

            def op_mul():
                st8["osb"] = spool.tile([P, SBW], BF16, tag="osb2", name="osb2")
                nc.vector.tensor_mul(st8["osb"][:], u01[:], st8["rcb"][:])

            def mk_proj(st):
                def op_proj():
                    sl = slice(st * P, (st + 1) * P)
                    py_t = aux.tile([P, D], FP32, tag="aux", name="pyt")
                    nc.tensor.matmul(
                        py_t[:], st8["osb"][:, sl], wo_b[:],
                        start=True, stop=True,
                    )
                    ysb = ypool.tile([P, D], FP32, tag="y", name="ysb")
                    if st < YSB_ON_ACT:
                        nc.scalar.copy(ysb[:], py_t[:])
                    else:
                        nc.vector.tensor_copy(ysb[:], py_t[:])
                    row = (sb * (SBW // P) + st) * P
                    nc.sync.dma_start(y_d[row : row + P, :], ysb[:])
                return op_proj

            epi_ops.extend(
                [op_bc, op_seed, op_t, op_u, op_rcb, op_mul]
                + [mk_proj(st) for st in range(SBW // P)]
            )
            pending[0] = None

        SPLICE = {}
        for _g in range(2, NSB):
            base = 2 + (_g - 2) * 4
            SPLICE[base] = lambda g=_g: produce_x_q(g)
            SPLICE[base + 1] = lambda g=_g: produce_k(g)
            SPLICE[base + 2] = lambda g=_g: produce_v_half(g, 0)
            SPLICE[base + 3] = lambda g=_g: produce_v_half(g, 1)
        NQ = NSB * NT
        assert NQ % 2 == 0 and PREF % 2 == 0
        po_cur = [None, None]
        ex_q = {}
        for qq in range(0, NQ + PREF, 2):
            qs = [q for q in (qq, qq + 1) if q < NQ]
            for q in qs:
                sb, tt = q // NT, q % NT
                if sb == 0 and tt in SPLICE:
                    SPLICE[tt]()
            # AV of PREF tiles ago is emitted BEFORE this pair's scores: its
            # inputs are long ready, so the in-order PE queue keeps
            # streaming even when the scores' exp dependency lags. Both AV
            # tiles are adjacent so the scores<->AV weight-load transition
            # is paid once per two tiles.
            for qa in (qq - PREF, qq - PREF + 1):
                if qa < 0:
                    continue
                sba, ta = qa // NT, qa % NT
                ga, ja = ta // GT, ta % GT
                if ta == 0:
                    po_cur[0] = pop.tile([65, SBW], FP32, tag="po", name="po0")
                    po_cur[1] = pop.tile([65, SBW], FP32, tag="po", name="po1")
                po0, po1 = po_cur
                vs, ex = v_g[ga], ex_q.pop(qa)
                nc.tensor.matmul(
                    po0[:],
                    vs[:, ja * 130 : ja * 130 + 65],
                    ex[:, 0:SBW],
                    start=(ta == 0),
                    stop=(ta == NT - 1),
                )
                nc.tensor.matmul(
                    po1[:],
                    vs[:, ja * 130 + 65 : ja * 130 + 130],
                    ex[:, SBW : 2 * SBW],
                    start=(ta == 0),
                    stop=(ta == NT - 1),
                )
                if ta == NT - 1:
                    epilogue_part1(sba, po0, po1)
            for q in qs:
                sb, tt = q // NT, q % NT
                g, j = tt // GT, tt % GT
                kt, qt = kT_g[g], qT_g[sb]
                ps_t = psp.tile([P, 2 * SBW], FP32, tag="ps")
                nc.tensor.matmul(
                    ps_t[:, 0:SBW],
                    kt[0:64, j * P : (j + 1) * P],
                    qt[0:64, :],
                    start=True,
                    stop=True,
                    tile_position=(0, 0),
                )
                nc.tensor.matmul(
                    ps_t[:, SBW : 2 * SBW],
                    kt[64:128, j * P : (j + 1) * P],
                    qt[64:128, :],
                    start=True,
                    stop=True,
                    tile_position=(64, 0),
                )
                ex = epool.tile([P, 2 * SBW], BF16, tag="exp")
                if (q % 16) in DVE_EXP_RESIDUES:
                    nc.vector.tensor_scalar(
                        ex[:].bitcast(I16), ps_t[:], EXP_A, EXP_B,
                        mybir.AluOpType.mult, mybir.AluOpType.add,
                    )
                else:
                    nc.scalar.activation(
                        ex[:], ps_t[:], mybir.ActivationFunctionType.Exp,
                        scale=float(1.0 / np.sqrt(DR)),
                    )
                ex_q[q] = ex
            if qq % NT == DEFER_ITERS:
                epilogue_part2a()
            if epi_ops:
                epi_ops.pop(0)()
        epilogue_part2a()
        while epi_ops:
            epi_ops.pop(0)()

    _split_sync_waits(nc)
    return nc


_program = None


def _get_program():
    global _program
    if _program is None:
        _program = _build_program()
    return _program


def _make_in_maps(x, Wq, Wk, Wv, Wo):
    import ml_dtypes

    bf16 = ml_dtypes.bfloat16
    xts = [np.ascontiguousarray(x[b].T).astype(bf16) for b in range(B)]
    in_maps = []
    for c in range(N_CORES):
        b = c // 4
        hp = c % 4
        h0, h1 = 2 * hp, 2 * hp + 1
        in_maps.append(
            {
                "xt": xts[b],
                "wq": np.ascontiguousarray(
                    np.concatenate([Wq[h0], Wq[h1]], axis=1)
                ).astype(bf16),
                "wk": np.ascontiguousarray(
                    np.concatenate([Wk[h0], Wk[h1]], axis=1)
                ).astype(bf16),
                "wv": np.ascontiguousarray(
                    np.concatenate([Wv[h0], Wv[h1]], axis=1)
                ).astype(bf16),
                "wo": np.ascontiguousarray(Wo[hp * 128 : (hp + 1) * 128]).astype(
                    bf16
                ),
            }
        )
    return in_maps


def kernel(**inputs) -> np.ndarray:
    x = np.asarray(inputs["x"], dtype=np.float32)
    Wq = np.asarray(inputs["Wq"], dtype=np.float32)
    Wk = np.asarray(inputs["Wk"], dtype=np.float32)
    Wv = np.asarray(inputs["Wv"], dtype=np.float32)
    Wo = np.asarray(inputs["Wo"], dtype=np.float32)
    bo = np.asarray(inputs["bo"], dtype=np.float32)

    nc = _get_program()
    in_maps = _make_in_maps(x, Wq, Wk, Wv, Wo)
    res = run_bass_kernel_spmd(nc, in_maps, list(range(N_CORES)))

    y = np.zeros((B, S, D), dtype=np.float32)
    for c in range(N_CORES):
        y[c // 4] += res.results[c]["y"]
    y += bo[None, None, :]
    return y


# revision 8
# speedup vs baseline: 1.0155x; 1.0070x over previous
"""Multi-head attention (B=2, S=4096, D=512, H=8, DR=64) on 8 trn2 NeuronCores.

Sharding: core c -> batch b = c // 4, head-pair hp = c % 4 (heads 2*hp, 2*hp+1).
Each core computes, for its batch and its two heads:
    q/k/v projections, flash-style attention (scores kept on-chip in
    transposed [t, s] orientation so softmax row-sums come from a fused
    ones-column in the AV matmul), and the partial output projection
    y_part = concat(out_h0, out_h1) @ Wo[rows of those heads].
Host sums the 4 partials per batch and adds the bias.

Matmul operands are cast to bf16 on-chip (fp32 matmuls on trn2 run as two
LOW/HIGH passes with an un-hidden LDWEIGHTS between them - ~3x the cost of a
bf16 matmul). All accumulation stays fp32 in PSUM; softmax denominators are
sums of the same quantized bf16 weights used in the AV matmul, keeping the
row-stochastic error common-mode.

Main-loop emission is batched two t-tiles at a time (scores pair-pair, then
the AV pairs of 2 earlier tiles): the AV stationary spans all 128 PE rows,
so its LDWEIGHTS cannot be pulled ahead past in-flight scores matmuls
(row-group conflict) and every scores<->AV transition exposes ~100ns of
weight-load; batching halves the transition count. The scores pair itself
runs concurrently on disjoint row groups (tile_position (0,0)/(64,0)), and
the per-head AV matmuls (M=65: 64 v-dims + the fused ones column) serialize
- 130 stationary columns don't fit the 128-wide array, which is structural.

The exp stream (33.5M elements/core, the largest engine load) is split
between the Scalar engine (exact table exp) and the Vector engine
(one-instruction Schraudolph: int16 bits of the bf16 result, ~2% rms rel
error whose row-sum component cancels against the fused denominator).
The per-block epilogue keeps the Vector queue shallow near block
boundaries: unnormalized-output evacuation is split ACT/DVE, the
reciprocal runs as an int16 bit-trick seed plus one bf16 Newton step, and
all epilogue ops are staged two t-tiles apart inside the next block's
t-loop so the in-order PE queue never stalls on engine latency.

The input pipeline (x load/cast + q/k/v projections) is emitted in 8 groups
of 512 s-columns; group-0/1 DMAs fan out across four engine queues so the
first projection matmul can issue ~4us earlier than a single-queue load.
"""

import sys

for _p in ("/opt/trn_rl_repo", "/root/.axon_site/_ro/trn_rl_repo"):
    if _p not in sys.path:
        sys.path.insert(0, _p)

import numpy as np
from contextlib import ExitStack

import concourse.bass as bass
import concourse.tile as tile
import concourse.mybir as mybir
from concourse.bass_utils import run_bass_kernel_spmd

B, S, D = 2, 4096, 512
H, DR = 8, 64
P = 128
NT = S // P          # 32 t-tiles (also s-tiles)
SBW = 512            # s-block width
NSB = S // SBW       # 8 s-blocks / t-groups
DC = D // P          # 4 d-chunks
GT = SBW // P        # 4 t-tiles per group
N_CORES = 8
FP32 = mybir.dt.float32
BF16 = mybir.dt.bfloat16
I16 = mybir.dt.int16

# exp offload: a subset of t-tiles compute exp on the Vector engine via a
# one-instruction Schraudolph (int16 bits of the bf16 result:
# bits = rint(score * EXP_A + EXP_B), bitcast to bf16 ~= exp(score/8) with
# ~2% rms relative error). The softmax denominator sums the same
# approximated values, so rows still sum to 1 and only the error
# *variation* reaches the output. Fraction tuned to balance ACT/DVE time
# against the tensor-engine roofline while keeping rel err << 2e-2.
EXP_A = float(128.0 / (np.sqrt(64.0) * np.log(2.0)))   # scale*128/ln2
EXP_B = float(127.0 * 128 - 5.625)                     # rint-optimal bias
RECIP_K = 32497.0   # bf16 reciprocal bit-trick magic (Newton-optimal)
DVE_EXP_RESIDUES = {1, 3, 5, 7, 9, 11, 13, 15}   # of q % 16 -> alpha = 1/2
# how many of the 4 per-block y copies run on the Scalar engine
YSB_ON_ACT = 4
PREF = 6             # AV lags scores by PREF t-tiles
DEFER_ITERS = 14     # epilogue part2 starts this many tiles into next block


# Kept as an extension point: this repo's walrus invocation hardcodes
# --enable-ldw-opt=false; flipping it to true crashes walrus codegen
# (visitInstLdweights), so the serialized weight-load cost is structural.
def _patch_ldw_opt():
    from concourse import bass_utils as _bu

    if getattr(_bu, "_ldw_opt_patched", False):
        return
    _bu._ldw_opt_patched = True
    _orig = _bu.run_command

    def patched_run(argv, **kwargs):
        return _orig(argv, **kwargs)

    _bu.run_command = patched_run


_patch_ldw_opt()

_drain_patched = False


def _patch_tile_drain():
    """This walrus build rejects >1 sync wait on one instruction, which breaks
    TileContext's kernel-tail drain. Spread the waits over nop instructions
    emitted just before the drain."""
    global _drain_patched
    if _drain_patched:
        return
    _drain_patched = True

    def patched(self, tick_clock, wait_clock):
        nop0 = self.nc.sync.nop()
        wait_clock.add_sem_waits(
            nop0.ins, tile.ScopedClock({None: tick_clock.global_clock})
        )
        si = nop0.ins.sync_info
        waits = list(si.on_wait) if si is not None else []
        if waits:
            nop0.ins.sync_info = mybir.SyncInfo(on_wait=waits[:1], on_update=[])
            for w in waits[1:]:
                nop = self.nc.sync.nop()
                nop.ins.sync_info = mybir.SyncInfo(on_wait=[w], on_update=[])
        self.nc.sync.drain()
        self.nc.all_engine_barrier()
        popped = self.nc._tile_sem_poison_stack.pop()
        assert popped is self._sem_poison
        self.nc.clear_and_free_semaphores(list(self.sems.allocated().values()))
        self.nc.all_engine_barrier()

    tile.TileContext._drain_and_barrier = patched


# This walrus build supports only one sync-wait slot per instruction, while
# Tile's sem-assigner attaches up to ~3. Spread the excess onto NoOp
# instructions inserted immediately before the owning instruction (same
# engine, so the stall point is identical and no deadlock can be introduced).
_WAIT_LIMIT = 1
_SKIP_OPCODES = {"AllEngineBarrier", "EventSemaphore", "Call"}


def _split_sync_waits(nc: bass.Bass):
    noop_cls = getattr(mybir, "InstNoOp", None)
    if noop_cls is None:
        import bass_rust

        noop_cls = bass_rust.InstNoOp
    counter = [0]
    for f in nc.m.functions:
        for blk in f.blocks:
            insts = blk.instructions
            new_list = []
            changed = False
            for inst in insts:
                si = inst.sync_info
                waits = list(si.on_wait) if si is not None and si.on_wait else []
                if (
                    len(waits) > _WAIT_LIMIT
                    and inst.opcode not in _SKIP_OPCODES
                    and all(w.sync_type == "semaphore" for w in waits)
                ):
                    excess = waits[: len(waits) - _WAIT_LIMIT]
                    keep = waits[len(waits) - _WAIT_LIMIT :]
                    for w in excess:
                        counter[0] += 1
                        new_list.append(
                            noop_cls(
                                name=f"I-waitsplit-{counter[0]}",
                                engine=inst.engine,
                                debug=inst.debug,
                                ins=[],
                                outs=[],
                                sync_info=mybir.SyncInfo(
                                    on_wait=[w], on_update=[]
                                ),
                            )
                        )
                    inst.sync_info = mybir.SyncInfo(
                        on_wait=keep, on_update=list(si.on_update or [])
                    )
                    changed = True
                new_list.append(inst)
            if changed:
                insts.clear()
                insts.extend(new_list)


def _build_program() -> bass.Bass:
    _patch_tile_drain()
    nc = bass.Bass()

    xt_d = nc.declare_dram_parameter("xt", [D, S], BF16, isOutput=False)
    wq_d = nc.declare_dram_parameter("wq", [D, P], BF16, isOutput=False)
    wk_d = nc.declare_dram_parameter("wk", [D, P], BF16, isOutput=False)
    wv_d = nc.declare_dram_parameter("wv", [D, P], BF16, isOutput=False)
    wo_d = nc.declare_dram_parameter("wo", [P, D], BF16, isOutput=False)
    y_d = nc.declare_dram_parameter("y", [S, D], FP32, isOutput=True)

    with tile.TileContext(nc) as tc, ExitStack() as ctx:
        consts = ctx.enter_context(tc.tile_pool(name="consts", bufs=1))
        wpool = ctx.enter_context(tc.tile_pool(name="weights", bufs=1))
        aux = ctx.enter_context(tc.tile_pool(name="aux", bufs=2, space="PSUM"))
        psp = ctx.enter_context(tc.tile_pool(name="ps", bufs=2, space="PSUM"))
        pop = ctx.enter_context(tc.tile_pool(name="po", bufs=2, space="PSUM"))
        epool = ctx.enter_context(tc.tile_pool(name="exp", bufs=10))
        spool = ctx.enter_context(tc.tile_pool(name="small", bufs=4))
        opool = ctx.enter_context(tc.tile_pool(name="osb", bufs=3))
        ypool = ctx.enter_context(tc.tile_pool(name="yout", bufs=3))

        # Weights in bf16; w*_b[p, c*128 + e] = W[c*128 + p, e].  wq first so
        # the first q-projection matmul can issue as early as possible.
        wq_b = wpool.tile([P, D], BF16)
        wk_b = wpool.tile([P, D], BF16)
        wv_b = wpool.tile([P, D], BF16)
        wo_b = wpool.tile([P, D], BF16)
        nc.gpsimd.dma_start(
            wq_b[:].rearrange("p (c e) -> p c e", c=DC),
            wq_d[:].rearrange("(c p) e -> p c e", p=P),
        )

        # Per-group persistent tiles (bufs=NSB so every group stays live).
        # xT_g[g][p, c*512 + j] = x[g*512 + j, c*128 + p]
        xtp = ctx.enter_context(tc.tile_pool(name="xtg", bufs=NSB))
        ktp = ctx.enter_context(tc.tile_pool(name="ktg", bufs=NSB))
        qtp = ctx.enter_context(tc.tile_pool(name="qtg", bufs=NSB))
        vsp = ctx.enter_context(tc.tile_pool(name="vsg", bufs=NSB))
        xT_g = [None] * NSB
        kT_g = [None] * NSB   # [e(h0|h1), 512 t-cols]
        qT_g = [None] * NSB   # [e(h0|h1), 512 s-cols]
        v_g = [None] * NSB    # per t-tile in group: [t, 65*2] = [vh0|1 | vh1|1]

        # group-0 x chunks fan out over four queues (prologue-idle engines)
        _G0_ENGINES = None

        def produce_x(g):
            xt = xtp.tile([P, DC * SBW], BF16, tag="xt")
            xT_g[g] = xt
            for c in range(DC):
                eng = (
                    _G0_ENGINES[c] if g == 0 else (nc.sync if c % 2 else nc.gpsimd)
                )
                eng.dma_start(
                    xt[:, c * SBW : (c + 1) * SBW],
                    xt_d[c * P : (c + 1) * P, g * SBW : (g + 1) * SBW],
                )

        def produce_q(g):
            xt = xT_g[g]
            qt = qtp.tile([P, SBW], BF16, tag="qt")
            qT_g[g] = qt
            pp = aux.tile([P, SBW], FP32, tag="aux")
            for c in range(DC):
                nc.tensor.matmul(
                    pp[:],
                    wq_b[:, c * P : (c + 1) * P],
                    xt[:, c * SBW : (c + 1) * SBW],
                    start=(c == 0),
                    stop=(c == DC - 1),
                )
            nc.vector.tensor_copy(qt[:], pp[:])

        def produce_x_q(g):
            produce_x(g)
            produce_q(g)

        def produce_k(g):
            xt = xT_g[g]
            kt = ktp.tile([P, SBW], BF16, tag="kt")
            kT_g[g] = kt
            pp = aux.tile([P, SBW], FP32, tag="aux")
            for c in range(DC):
                nc.tensor.matmul(
                    pp[:],
                    wk_b[:, c * P : (c + 1) * P],
                    xt[:, c * SBW : (c + 1) * SBW],
                    start=(c == 0),
                    stop=(c == DC - 1),
                )
            nc.vector.tensor_copy(kt[:], pp[:])

        def produce_v_half(g, half):
            xt = xT_g[g]
            if half == 0:
                vs = vsp.tile([P, GT * 130], BF16, tag="vs")
                v_g[g] = vs
            else:
                vs = v_g[g]
            for j in (0, 1) if half == 0 else (2, 3):
                pv = aux.tile([P, P], FP32, tag="aux")
                for c in range(DC):
                    nc.tensor.matmul(
                        pv[:],
                        xt[:, c * SBW + j * P : c * SBW + (j + 1) * P],
                        wv_b[:, c * P : (c + 1) * P],
                        start=(c == 0),
                        stop=(c == DC - 1),
                    )
                dstv = vs[:, j * 130 : j * 130 + 130].rearrange(
                    "p (h q) -> p h q", h=2
                )[:, :, 0:64]
                nc.vector.tensor_copy(
                    dstv, pv[:].rearrange("p (h q) -> p h q", h=2)
                )
            if half == 1:
                ones_cols = vs[:].rearrange("p (t q) -> p t q", t=GT)[
                    :, :, 64:130:65
                ]
                nc.vector.memset(ones_cols, 1.0)

        # ---- prologue: group 0/1 inputs + weights, maximally parallel ----
        _G0_ENGINES = [nc.sync, nc.scalar, nc.sync, nc.gpsimd]
        produce_x(0)
        nc.scalar.dma_start(
            wk_b[:].rearrange("p (c e) -> p c e", c=DC),
            wk_d[:].rearrange("(c p) e -> p c e", p=P),
        )
        produce_x(1)
        nc.scalar.dma_start(
            wv_b[:].rearrange("p (c e) -> p c e", c=DC),
            wv_d[:].rearrange("(c p) e -> p c e", p=P),
        )
        nc.scalar.dma_start(wo_b[:], wo_d[:])

        ones64 = consts.tile([1, 64], BF16)
        nc.vector.memset(ones64[:], 1.0)

        produce_q(0)
        produce_k(0)
        produce_v_half(0, 0)
        produce_v_half(0, 1)
        produce_q(1)
        produce_k(1)
        produce_v_half(1, 0)
        produce_v_half(1, 1)

        # ---- attention + output projection ----
        # Epilogue part 1 (right after a block's t-loop): evacuate both heads'
        # unnormalized output into one [128, 512] bf16 tile (h1 at partitions
        # 64..127, aligned with the broadcast slice used by the normalize
        # mul), plus the two softmax-sum rows. Part 2 (deferred into the next
        # block's t-loop): broadcast the sums to 128 partitions with two tiny
        # col-tiled matmuls, reciprocal via int16 bit-trick seed + one bf16
        # Newton step, one normalize mul, then the four projection matmuls.
        pending = [None]

        def epilogue_part1(sb, po0, po1):
            u01 = opool.tile([P, SBW], BF16, tag="u01")
            nc.scalar.copy(u01[0:64, :], po0[0:64, :])
            nc.vector.tensor_copy(u01[64:128, :], po1[0:64, :])
            # sums rows to partition-0 tiles (matmul rhs must start at 0)
            s0 = spool.tile([1, SBW], BF16, tag="s0")
            s1 = spool.tile([1, SBW], BF16, tag="s1")
            nc.vector.tensor_copy(s0[:], po0[64:65, :])
            nc.vector.tensor_copy(s1[:], po1[64:65, :])
            pending[0] = (sb, u01, s0, s1)

        epi_ops = []

        def epilogue_part2a():
            if pending[0] is None:
                return
            sb, u01, s0, s1 = pending[0]
            # broadcast the bf16 sums rows to 128 partitions (1-pass bf16 MMs)
            pb_t = aux.tile([P, SBW], FP32, tag="aux")
            nc.tensor.matmul(
                pb_t[0:64, :], ones64[:], s0[:],
                start=True, stop=True, tile_position=(0, 0),
            )
            nc.tensor.matmul(
                pb_t[64:128, :], ones64[:], s1[:],
                start=True, stop=True, tile_position=(0, 64),
            )
            st8 = {}

            def op_bc():
                st8["bc"] = spool.tile([P, SBW], BF16, tag="bc", name="bc")
                nc.scalar.copy(st8["bc"][:], pb_t[:])

            def op_seed():
                st8["rs"] = spool.tile([P, SBW], BF16, tag="rs", name="rs")
                nc.vector.tensor_scalar(
                    st8["rs"][:].bitcast(I16), st8["bc"][:].bitcast(I16),
                    -1.0, float(RECIP_K),
                    mybir.AluOpType.mult, mybir.AluOpType.add,
                )

            def op_t():
                st8["tt"] = spool.tile([P, SBW], BF16, tag="tt", name="tt")
                nc.vector.tensor_mul(st8["tt"][:], st8["bc"][:], st8["rs"][:])

            def op_u():
                st8["uu"] = spool.tile([P, SBW], BF16, tag="uu", name="uu")
                nc.vector.tensor_scalar(
                    st8["uu"][:], st8["tt"][:], -1.0, 2.0,
                    mybir.AluOpType.mult, mybir.AluOpType.add,
                )

            def op_rcb():
                st8["rcb"] = spool.tile([P, SBW], BF16, tag="rcb", name="rcb")
                nc.vector.tensor_mul(st8["rcb"][:], st8["rs"][:], st8["uu"][:])

            def op_mul():
                st8["osb"] = spool.tile([P, SBW], BF16, tag="osb2", name="osb2")
                nc.vector.tensor_mul(st8["osb"][:], u01[:], st8["rcb"][:])

            def mk_proj(st_pair):
                def op_proj():
                    # two adjacent projection matmuls per pop so the
                    # proj<->attention weight-load transition is paid once
                    for st in st_pair:
                        sl = slice(st * P, (st + 1) * P)
                        py_t = aux.tile([P, D], FP32, tag="aux", name="pyt")
                        nc.tensor.matmul(
                            py_t[:], st8["osb"][:, sl], wo_b[:],
                            start=True, stop=True,
                        )
                        ysb = ypool.tile([P, D], FP32, tag="y", name="ysb")
                        if st < YSB_ON_ACT:
                            nc.scalar.copy(ysb[:], py_t[:])
                        else:
                            nc.vector.tensor_copy(ysb[:], py_t[:])
                        row = (sb * (SBW // P) + st) * P
                        nc.sync.dma_start(y_d[row : row + P, :], ysb[:])
                return op_proj

            epi_ops.extend(
                [op_bc, op_seed, op_t, op_u, op_rcb, op_mul]
                + [mk_proj((0, 1)), mk_proj((2, 3))]
            )
            pending[0] = None

        SPLICE = {}
        for _g in range(2, NSB):
            base = 2 + (_g - 2) * 4
            SPLICE[base] = lambda g=_g: produce_x_q(g)
            SPLICE[base + 1] = lambda g=_g: produce_k(g)
            SPLICE[base + 2] = lambda g=_g: produce_v_half(g, 0)
            SPLICE[base + 3] = lambda g=_g: produce_v_half(g, 1)
        NQ = NSB * NT
        assert NQ % 2 == 0 and PREF % 2 == 0
        po_cur = [None, None]
        ex_q = {}
        for qq in range(0, NQ + PREF, 2):
            qs = [q for q in (qq, qq + 1) if q < NQ]
            for q in qs:
                sb, tt = q // NT, q % NT
                if sb == 0 and tt in SPLICE:
                    SPLICE[tt]()
            # AV of PREF tiles ago is emitted BEFORE this pair's scores: its
            # inputs are long ready, so the in-order PE queue keeps
            # streaming even when the scores' exp dependency lags. Both AV
            # tiles are adjacent so the scores<->AV weight-load transition
            # is paid once per two tiles.
            for qa in (qq - PREF, qq - PREF + 1):
                if qa < 0:
                    continue
                sba, ta = qa // NT, qa % NT
                ga, ja = ta // GT, ta % GT
                if ta == 0:
                    po_cur[0] = pop.tile([65, SBW], FP32, tag="po", name="po0")
                    po_cur[1] = pop.tile([65, SBW], FP32, tag="po", name="po1")
                po0, po1 = po_cur
                vs, ex = v_g[ga], ex_q.pop(qa)
                nc.tensor.matmul(
                    po0[:],
                    vs[:, ja * 130 : ja * 130 + 65],
                    ex[:, 0:SBW],
                    start=(ta == 0),
                    stop=(ta == NT - 1),
                )
                nc.tensor.matmul(
                    po1[:],
                    vs[:, ja * 130 + 65 : ja * 130 + 130],
                    ex[:, SBW : 2 * SBW],
                    start=(ta == 0),
                    stop=(ta == NT - 1),
                )
                if ta == NT - 1:
                    epilogue_part1(sba, po0, po1)
            for q in qs:
                sb, tt = q // NT, q % NT
                g, j = tt // GT, tt % GT
                kt, qt = kT_g[g], qT_g[sb]
                ps_t = psp.tile([P, 2 * SBW], FP32, tag="ps")
                nc.tensor.matmul(
                    ps_t[:, 0:SBW],
                    kt[0:64, j * P : (j + 1) * P],
                    qt[0:64, :],
                    start=True,
                    stop=True,
                    tile_position=(0, 0),
                )
                nc.tensor.matmul(
                    ps_t[:, SBW : 2 * SBW],
                    kt[64:128, j * P : (j + 1) * P],
                    qt[64:128, :],
                    start=True,
                    stop=True,
                    tile_position=(64, 0),
                )
                ex = epool.tile([P, 2 * SBW], BF16, tag="exp")
                if (q % 16) in DVE_EXP_RESIDUES:
                    nc.vector.tensor_scalar(
                        ex[:].bitcast(I16), ps_t[:], EXP_A, EXP_B,
                        mybir.AluOpType.mult, mybir.AluOpType.add,
                    )
                else:
                    nc.scalar.activation(
                        ex[:], ps_t[:], mybir.ActivationFunctionType.Exp,
                        scale=float(1.0 / np.sqrt(DR)),
                    )
                ex_q[q] = ex
            if qq % NT == DEFER_ITERS:
                epilogue_part2a()
            if epi_ops:
                epi_ops.pop(0)()
        epilogue_part2a()
        while epi_ops:
            epi_ops.pop(0)()

    _split_sync_waits(nc)
    return nc


_program = None


def _get_program():
    global _program
    if _program is None:
        _program = _build_program()
    return _program


def _make_in_maps(x, Wq, Wk, Wv, Wo):
    import ml_dtypes

    bf16 = ml_dtypes.bfloat16
    xts = [np.ascontiguousarray(x[b].T).astype(bf16) for b in range(B)]
    in_maps = []
    for c in range(N_CORES):
        b = c // 4
        hp = c % 4
        h0, h1 = 2 * hp, 2 * hp + 1
        in_maps.append(
            {
                "xt": xts[b],
                "wq": np.ascontiguousarray(
                    np.concatenate([Wq[h0], Wq[h1]], axis=1)
                ).astype(bf16),
                "wk": np.ascontiguousarray(
                    np.concatenate([Wk[h0], Wk[h1]], axis=1)
                ).astype(bf16),
                "wv": np.ascontiguousarray(
                    np.concatenate([Wv[h0], Wv[h1]], axis=1)
                ).astype(bf16),
                "wo": np.ascontiguousarray(Wo[hp * 128 : (hp + 1) * 128]).astype(
                    bf16
                ),
            }
        )
    return in_maps


def kernel(**inputs) -> np.ndarray:
    x = np.asarray(inputs["x"], dtype=np.float32)
    Wq = np.asarray(inputs["Wq"], dtype=np.float32)
    Wk = np.asarray(inputs["Wk"], dtype=np.float32)
    Wv = np.asarray(inputs["Wv"], dtype=np.float32)
    Wo = np.asarray(inputs["Wo"], dtype=np.float32)
    bo = np.asarray(inputs["bo"], dtype=np.float32)

    nc = _get_program()
    in_maps = _make_in_maps(x, Wq, Wk, Wv, Wo)
    res = run_bass_kernel_spmd(nc, in_maps, list(range(N_CORES)))

    y = np.zeros((B, S, D), dtype=np.float32)
    for c in range(N_CORES):
        y[c // 4] += res.results[c]["y"]
    y += bo[None, None, :]
    return y
